# revision 5
# baseline (speedup 1.0000x reference)
"""Trainium2 Bass kernel for nn_AdaptiveHyperNN (gnn_message_passing).

Math: the reference builds fully-connected edge features [B,N,N,D] through
linear layers, then mean-aggregates.  Because every edge MLP is linear
(no nonlinearity before aggregation), everything collapses:

  feat   = api_embeds[invoked]                       [B,N,D]
  e2n1_v = fbar @ W1a + feat_v @ W1b + b1            (fbar = mean_v feat_v)
  h_v    = feat_v @ W2a + e2n1_v @ W2b + b2
  logit[u*N+v] = p[u] + q[v] + c
     p = h @ (W3a @ W4a),  q = h @ (W3b @ W4a)
     c = b3 @ W4a + Xs @ W4b + b4
  out = sigmoid(logit)

Sharding: data-parallel over B (8 graphs -> 8 cores), weights replicated.
Each core: indirect-DMA gather of 128 embedding rows, ~16 [128x128x128]
matmuls, one rank-2 outer-sum matmul, sigmoid, DMA out 64KB.
"""

import numpy as np

import concourse.bass as bass
import concourse.bacc as bacc
import concourse.mybir as mybir
import concourse.tile as tile
from concourse.bass import ts
from concourse.bass_utils import run_bass_kernel_spmd
from concourse.masks import make_identity

P = 128
D = 256
N = 128
B = 8
V = 10000
F32 = mybir.dt.float32
I32 = mybir.dt.int32

TRACE = False
LAST_RESULTS = None

_NC_CACHE = {}


def _build_nc():
    nc = bacc.Bacc("TRN2", target_bir_lowering=False)

    inv = nc.dram_tensor("invoked", [N, 1], I32, kind="ExternalInput")
    emb = nc.dram_tensor("emb", [V, D], F32, kind="ExternalInput")
    w1 = nc.dram_tensor("w1", [2 * D, D], F32, kind="ExternalInput")
    w2 = nc.dram_tensor("w2", [2 * D, D], F32, kind="ExternalInput")
    w3t = nc.dram_tensor("w3t", [D, 2 * D], F32, kind="ExternalInput")
    w4 = nc.dram_tensor("w4", [2 * D, 1], F32, kind="ExternalInput")
    b1 = nc.dram_tensor("b1", [D, 1], F32, kind="ExternalInput")
    b2 = nc.dram_tensor("b2", [D, 1], F32, kind="ExternalInput")
    b3 = nc.dram_tensor("b3", [D, 1], F32, kind="ExternalInput")
    b4 = nc.dram_tensor("b4", [1, 1], F32, kind="ExternalInput")
    xs = nc.dram_tensor("xs", [D, 1], F32, kind="ExternalInput")
    out = nc.dram_tensor("out", [N * N, 1], F32, kind="ExternalOutput")

    KT = (2 * D) // P  # 4 k-tiles over the 2D=512 axis
    DT = D // P        # 2 tiles over the D=256 axis

    with tile.TileContext(nc) as tc:
        with (
            tc.tile_pool(name="sb", bufs=1) as sb,
            tc.tile_pool(name="ps", bufs=2, space="PSUM") as ps,
        ):
            # ---- load weights/consts ----
            w1_t = [sb.tile([P, D], F32, tag=f"w1_{k}", name=f"w1_{k}") for k in range(KT)]
            w2_t = [sb.tile([P, D], F32, tag=f"w2_{k}", name=f"w2_{k}") for k in range(KT)]
            for k in range(KT):
                nc.sync.dma_start(out=w1_t[k][:], in_=w1[ts(k, P), :])
                nc.sync.dma_start(out=w2_t[k][:], in_=w2[ts(k, P), :])
            w3t_t = [sb.tile([P, 2 * D], F32, tag=f"w3t_{k}", name=f"w3t_{k}") for k in range(DT)]
            for k in range(DT):
                nc.sync.dma_start(out=w3t_t[k][:], in_=w3t[ts(k, P), :])
            w4_t = [sb.tile([P, 1], F32, tag=f"w4_{k}", name=f"w4_{k}") for k in range(KT)]
            for k in range(KT):
                nc.sync.dma_start(out=w4_t[k][:], in_=w4[ts(k, P), :])
            b1_t = [sb.tile([P, 1], F32, tag=f"b1_{k}", name=f"b1_{k}") for k in range(DT)]
            b2_t = [sb.tile([P, 1], F32, tag=f"b2_{k}", name=f"b2_{k}") for k in range(DT)]
            b3_t = [sb.tile([P, 1], F32, tag=f"b3_{k}", name=f"b3_{k}") for k in range(DT)]
            xs_t = [sb.tile([P, 1], F32, tag=f"xs_{k}", name=f"xs_{k}") for k in range(DT)]
            for k in range(DT):
                nc.sync.dma_start(out=b1_t[k][:], in_=b1[ts(k, P), :])
                nc.sync.dma_start(out=b2_t[k][:], in_=b2[ts(k, P), :])
                nc.sync.dma_start(out=b3_t[k][:], in_=b3[ts(k, P), :])
                nc.sync.dma_start(out=xs_t[k][:], in_=xs[ts(k, P), :])
            b4_t = sb.tile([1, 1], F32, tag="b4", name="b4")
            nc.sync.dma_start(out=b4_t[:], in_=b4[:, :])
            inv_t = sb.tile([P, 1], I32, tag="inv", name="inv")
            nc.sync.dma_start(out=inv_t[:], in_=inv[:, :])

            ident = sb.tile([P, P], F32, tag="ident", name="ident")
            make_identity(nc, ident[:])

            # ---- gather feat = emb[invoked]  [128 nodes, 256] ----
            feat = sb.tile([P, D], F32, tag="feat", name="feat")
            nc.gpsimd.indirect_dma_start(
                out=feat[:],
                out_offset=None,
                in_=emb[:, :],
                in_offset=bass.IndirectOffsetOnAxis(ap=inv_t[:, :1], axis=0),
            )

            # ---- transpose -> featT tiles [din, node], and fbar sum ----
            featT = [sb.tile([P, P], F32, tag=f"featT_{i}", name=f"featT_{i}") for i in range(DT)]
            fbar = [sb.tile([P, 1], F32, tag=f"fbar_{i}", name=f"fbar_{i}") for i in range(DT)]
            for i in range(DT):
                ptr = ps.tile([P, P], F32, tag="big", name="big")
                nc.tensor.transpose(out=ptr[:], in_=feat[:, ts(i, P)], identity=ident[:])
                nc.vector.tensor_copy(out=featT[i][:], in_=ptr[:])
                nc.vector.reduce_sum(
                    out=fbar[i][:], in_=featT[i][:], axis=mybir.AxisListType.X
                )

            # ---- e_bias = (fbar_sum @ W1a)/128 + b1   [dout,1] per tile ----
            e_bias = [sb.tile([P, 1], F32, tag=f"ebias_{m}", name=f"ebias_{m}") for m in range(DT)]
            for mt in range(DT):
                pt1 = ps.tile([P, 1], F32, tag="small", name="small")
                for kt in range(DT):
                    nc.tensor.matmul(
                        out=pt1[:],
                        lhsT=w1_t[kt][:, ts(mt, P)],
                        rhs=fbar[kt][:],
                        start=(kt == 0),
                        stop=(kt == DT - 1),
                    )
                nc.scalar.activation(
                    out=e_bias[mt][:],
                    in_=pt1[:],
                    func=mybir.ActivationFunctionType.Identity,
                    bias=b1_t[mt][:, :1],
                    scale=1.0 / N,
                )

            # ---- e2n1T[dout, node] = W1b.T-contract featT + e_bias ----
            e2n1T = [sb.tile([P, P], F32, tag=f"e2n1T_{m}", name=f"e2n1T_{m}") for m in range(DT)]
            for mt in range(DT):
                pe = ps.tile([P, P], F32, tag="big", name="big")
                for kt in range(DT):
                    nc.tensor.matmul(
                        out=pe[:],
                        lhsT=w1_t[DT + kt][:, ts(mt, P)],
                        rhs=featT[kt][:],
                        start=(kt == 0),
                        stop=(kt == DT - 1),
                    )
                nc.scalar.activation(
                    out=e2n1T[mt][:],
                    in_=pe[:],
                    func=mybir.ActivationFunctionType.Identity,
                    bias=e_bias[mt][:, :1],
                )

            # ---- hT[dout, node] = W2a.T featT + W2b.T e2n1T + b2 ----
            hT = [sb.tile([P, P], F32, tag=f"hT_{m}", name=f"hT_{m}") for m in range(DT)]
            for mt in range(DT):
                ph = ps.tile([P, P], F32, tag="big", name="big")
                for kt in range(DT):
                    nc.tensor.matmul(
                        out=ph[:],
                        lhsT=w2_t[kt][:, ts(mt, P)],
                        rhs=featT[kt][:],
                        start=(kt == 0),
                        stop=False,
                    )
                for kt in range(DT):
                    nc.tensor.matmul(
                        out=ph[:],
                        lhsT=w2_t[DT + kt][:, ts(mt, P)],
                        rhs=e2n1T[kt][:],
                        start=False,
                        stop=(kt == DT - 1),
                    )
                nc.scalar.activation(
                    out=hT[mt][:],
                    in_=ph[:],
                    func=mybir.ActivationFunctionType.Identity,
                    bias=b2_t[mt][:, :1],
                )

            # ---- w34 = W3 @ W4a  (512-vector), packed as [d,2] per d-tile ----
            w34cat = [sb.tile([P, 2], F32, tag=f"w34c_{d}", name=f"w34c_{d}") for d in range(DT)]
            for it in range(KT):
                pw = ps.tile([P, 1], F32, tag="small", name="small")
                for jt in range(DT):
                    nc.tensor.matmul(
                        out=pw[:],
                        lhsT=w3t_t[jt][:, ts(it, P)],
                        rhs=w4_t[jt][:],
                        start=(jt == 0),
                        stop=(jt == DT - 1),
                    )
                d, col = it % DT, it // DT
                nc.vector.tensor_copy(out=w34cat[d][:, col : col + 1], in_=pw[:])

            # ---- c = b3 @ W4a + Xs @ W4b + b4   (scalar) ----
            pc = ps.tile([1, 1], F32, tag="small", name="small")
            nc.tensor.matmul(out=pc[:], lhsT=b3_t[0][:], rhs=w4_t[0][:], start=True, stop=False)
            nc.tensor.matmul(out=pc[:], lhsT=b3_t[1][:], rhs=w4_t[1][:], start=False, stop=False)
            nc.tensor.matmul(out=pc[:], lhsT=xs_t[0][:], rhs=w4_t[2][:], start=False, stop=False)
            nc.tensor.matmul(out=pc[:], lhsT=xs_t[1][:], rhs=w4_t[3][:], start=False, stop=True)
            c_sb = sb.tile([1, 1], F32, tag="c_sb", name="c_sb")
            nc.vector.tensor_add(out=c_sb[:], in0=pc[:], in1=b4_t[:])

            # ---- p/q rows on partition 0 (quadrant rule forbids partition
            #      offsets for compute engines) ----
            p_ps = ps.tile([1, P], F32, tag="small", name="small")
            q_ps = ps.tile([1, P], F32, tag="small2", name="small2")
            for d in range(DT):
                nc.tensor.matmul(
                    out=p_ps[:],
                    lhsT=w34cat[d][:, 0:1],
                    rhs=hT[d][:],
                    start=(d == 0),
                    stop=(d == DT - 1),
                )
            for d in range(DT):
                nc.tensor.matmul(
                    out=q_ps[:],
                    lhsT=w34cat[d][:, 1:2],
                    rhs=hT[d][:],
                    start=(d == 0),
                    stop=(d == DT - 1),
                )

            # ---- outer sum via two rank-1 matmuls, then sigmoid ----
            p_row = sb.tile([1, P], F32, tag="p_row", name="p_row")
            q_row = sb.tile([1, P], F32, tag="q_row", name="q_row")
            ones_row = sb.tile([1, P], F32, tag="ones_row", name="ones_row")
            nc.vector.tensor_scalar_add(
                out=p_row[:], in0=p_ps[:], scalar1=c_sb[:1, :1]
            )
            nc.vector.tensor_copy(out=q_row[:], in_=q_ps[:])
            nc.vector.memset(ones_row[:], 1.0)

            pz = ps.tile([P, P], F32, tag="big", name="big")
            nc.tensor.matmul(out=pz[:], lhsT=p_row[:], rhs=ones_row[:], start=True, stop=False)
            nc.tensor.matmul(out=pz[:], lhsT=ones_row[:], rhs=q_row[:], start=False, stop=True)

            osb = sb.tile([P, P], F32, tag="osb", name="osb")
            nc.scalar.activation(
                out=osb[:], in_=pz[:], func=mybir.ActivationFunctionType.Sigmoid
            )
            nc.sync.dma_start(
                out=out[:, :].rearrange("(u v) o -> u (v o)", v=N), in_=osb[:]
            )

    nc.compile()
    return nc


def kernel(Xs, api_embeds, W1, b1, W2, b2, W3, b3, W4, b4, invoked):
    global LAST_RESULTS
    if "nc" not in _NC_CACHE:
        _NC_CACHE["nc"] = _build_nc()
    nc = _NC_CACHE["nc"]

    Xs = np.ascontiguousarray(np.asarray(Xs, dtype=np.float32))
    emb = np.ascontiguousarray(np.asarray(api_embeds, dtype=np.float32))
    W1 = np.ascontiguousarray(np.asarray(W1, dtype=np.float32))
    W2 = np.ascontiguousarray(np.asarray(W2, dtype=np.float32))
    W3t = np.ascontiguousarray(np.asarray(W3, dtype=np.float32).T)
    W4 = np.ascontiguousarray(np.asarray(W4, dtype=np.float32).reshape(2 * D, 1))
    b1 = np.ascontiguousarray(np.asarray(b1, dtype=np.float32).reshape(D, 1))
    b2 = np.ascontiguousarray(np.asarray(b2, dtype=np.float32).reshape(D, 1))
    b3 = np.ascontiguousarray(np.asarray(b3, dtype=np.float32).reshape(D, 1))
    b4 = np.ascontiguousarray(np.asarray(b4, dtype=np.float32).reshape(1, 1))
    invoked = np.asarray(invoked, dtype=np.int32)

    in_maps = []
    for b in range(B):
        in_maps.append(
            {
                "invoked": np.ascontiguousarray(invoked[b].reshape(N, 1)),
                "emb": emb,
                "w1": W1,
                "w2": W2,
                "w3t": W3t,
                "w4": W4,
                "b1": b1,
                "b2": b2,
                "b3": b3,
                "b4": b4,
                "xs": np.ascontiguousarray(Xs[b].reshape(D, 1)),
            }
        )

    res = run_bass_kernel_spmd(nc, in_maps, core_ids=list(range(B)), trace=TRACE)
    LAST_RESULTS = res
    return np.stack([res.results[i]["out"] for i in range(B)], axis=0)


# revision 7
# speedup vs baseline: 1.3610x; 1.3610x over previous
"""Trainium2 Bass kernel for nn_AdaptiveHyperNN (gnn_message_passing).

Math: the reference builds fully-connected edge features [B,N,N,D] through
linear layers, then mean-aggregates.  Because every edge MLP is linear
(no nonlinearity before aggregation), everything collapses:

  feat   = api_embeds[invoked]                       [B,N,D]
  e2n1_v = fbar @ W1a + feat_v @ W1b + b1            (fbar = mean_v feat_v)
  h_v    = feat_v @ W2a + e2n1_v @ W2b + b2
  logit[u*N+v] = p[u] + q[v] + c
     p = h @ (W3a @ W4a),  q = h @ (W3b @ W4a)
     c = b3 @ W4a + Xs @ W4b + b4
  out = sigmoid(logit)

Sharding: data-parallel over B (8 graphs -> 8 cores), weights replicated.
Each core: indirect-DMA gather of 128 embedding rows, ~16 [128x128x128]
matmuls, rank-1 outer-sum matmuls, sigmoid, DMA out 64KB.

Perf structure: all weights/consts are host-packed into one [128, 3085]
DRAM tensor, loaded by 4 large DMAs split across the two HWDGE rings
(sync + scalar engines).  The int32 `invoked` indices go first on the
scalar ring so the gpsimd indirect gather starts immediately.  PSUM
evictions ride the vector engine (tensor_scalar) so the scalar engine
only ever runs Sigmoid (single ACT table load, warmed up early).
"""

import numpy as np

import concourse.bacc as bacc
import concourse.bass as bass
import concourse.mybir as mybir
import concourse.tile as tile
from concourse.bass import ts
from concourse.bass_utils import run_bass_kernel_spmd
from concourse.masks import make_identity

P = 128
D = 256
N = 128
B = 8
V = 10000
F32 = mybir.dt.float32
I32 = mybir.dt.int32

# packed column layout
_W1 = 0       # 4 k-tiles x 256
_W2 = 1024    # 4 k-tiles x 256
_W3T = 2048   # 2 j-tiles x 512
_W4 = 3072    # 4 cols
_B1 = 3076    # 2 cols
_B2 = 3078
_B3 = 3080
_XS = 3082
_B4 = 3084
PKC = 3085

TRACE = False
LAST_RESULTS = None

_NC_CACHE = {}


def _build_nc():
    nc = bacc.Bacc("TRN2", target_bir_lowering=False)

    inv = nc.dram_tensor("invoked", [N, 1], I32, kind="ExternalInput")
    emb = nc.dram_tensor("emb", [V, D], F32, kind="ExternalInput")
    pk = nc.dram_tensor("pk", [P, PKC], F32, kind="ExternalInput")
    out = nc.dram_tensor("out", [N * N, 1], F32, kind="ExternalOutput")

    KT = (2 * D) // P  # 4 k-tiles over the 2D=512 axis
    DT = D // P        # 2 tiles over the D=256 axis

    TS = mybir.ActivationFunctionType

    with tile.TileContext(nc) as tc:
        with (
            tc.tile_pool(name="sb", bufs=1) as sb,
            tc.tile_pool(name="ps", bufs=1, space="PSUM") as ps,
        ):
            # ---- indices first (scalar/ACT ring), gather ASAP ----
            inv_t = sb.tile([P, 1], I32, tag="inv", name="inv")
            nc.scalar.dma_start(out=inv_t[:], in_=inv[:, :])

            # sigmoid ACT-table warmup off the critical path
            warm = sb.tile([1, 1], F32, tag="warm", name="warm")
            warm2 = sb.tile([1, 1], F32, tag="warm2", name="warm2")
            nc.vector.memset(warm[:], 0.0)
            nc.scalar.activation(out=warm2[:], in_=warm[:], func=TS.Sigmoid)

            ident = sb.tile([P, P], F32, tag="ident", name="ident")
            make_identity(nc, ident[:])

            # ---- gather feat = emb[invoked]  [128 nodes, 256] ----
            feat = sb.tile([P, D], F32, tag="feat", name="feat")
            nc.gpsimd.indirect_dma_start(
                out=feat[:],
                out_offset=None,
                in_=emb[:, :],
                in_offset=bass.IndirectOffsetOnAxis(ap=inv_t[:, :1], axis=0),
            )

            # ---- packed weights: 4 large DMAs over the two HWDGE rings ----
            sm_sb = sb.tile([P, PKC - _W4], F32, tag="sm", name="sm")
            wa_sb = sb.tile([P, 1024], F32, tag="wa", name="wa")
            wb_sb = sb.tile([P, 1024], F32, tag="wb", name="wb")
            wc_sb = sb.tile([P, 1024], F32, tag="wc", name="wc")
            nc.scalar.dma_start(out=sm_sb[:], in_=pk[:, _W4:PKC])
            nc.sync.dma_start(out=wa_sb[:], in_=pk[:, _W1:_W1 + 1024])
            nc.scalar.dma_start(out=wb_sb[:], in_=pk[:, _W2:_W2 + 1024])
            nc.sync.dma_start(out=wc_sb[:], in_=pk[:, _W3T:_W3T + 1024])

            def w1s(kt, mt):
                return wa_sb[:, kt * 256 + mt * 128 : kt * 256 + (mt + 1) * 128]

            def w2s(kt, mt):
                return wb_sb[:, kt * 256 + mt * 128 : kt * 256 + (mt + 1) * 128]

            def w3s(jt, it):
                return wc_sb[:, jt * 512 + it * 128 : jt * 512 + (it + 1) * 128]

            def w4s(kt):
                return sm_sb[:, kt : kt + 1]

            def b1s(k):
                return sm_sb[:, 4 + k : 5 + k]

            def b2s(k):
                return sm_sb[:, 6 + k : 7 + k]

            def b3s(k):
                return sm_sb[:, 8 + k : 9 + k]

            def xss(k):
                return sm_sb[:, 10 + k : 11 + k]

            b4s = sm_sb[0:1, 12:13]

            ones_row = sb.tile([1, P], F32, tag="ones_row", name="ones_row")
            nc.vector.memset(ones_row[:], 1.0)

            # ---- transpose -> featT tiles [din, node]; fbar = row sums ----
            featT = [sb.tile([P, P], F32, tag=f"featT_{i}", name=f"featT_{i}") for i in range(DT)]
            fbar = [sb.tile([P, 1], F32, tag=f"fbar_{i}", name=f"fbar_{i}") for i in range(DT)]
            for i in range(DT):
                ptr = ps.tile([P, P], F32, tag="big", name="big", bufs=3)
                nc.tensor.transpose(out=ptr[:], in_=feat[:, ts(i, P)], identity=ident[:])
                nc.vector.tensor_copy(out=featT[i][:], in_=ptr[:])
                nc.vector.reduce_sum(
                    out=fbar[i][:], in_=ptr[:], axis=mybir.AxisListType.X
                )

            # ---- e_bias = (fbar_sum @ W1a)/128 + b1   [dout,1] per tile ----
            e_bias = [sb.tile([P, 1], F32, tag=f"ebias_{m}", name=f"ebias_{m}") for m in range(DT)]
            for mt in range(DT):
                pt1 = ps.tile([P, 1], F32, tag="small", name="small", bufs=2)
                for kt in range(DT):
                    nc.tensor.matmul(
                        out=pt1[:],
                        lhsT=w1s(kt, mt),
                        rhs=fbar[kt][:],
                        start=(kt == 0),
                        stop=(kt == DT - 1),
                    )
                nc.vector.tensor_scalar(
                    out=e_bias[mt][:],
                    in0=pt1[:],
                    scalar1=1.0 / N,
                    scalar2=b1s(mt),
                    op0=mybir.AluOpType.mult,
                    op1=mybir.AluOpType.add,
                )

            # ---- e2n1T[dout, node] = W1b.T-contract featT + e_bias ----
            e2n1T = [sb.tile([P, P], F32, tag=f"e2n1T_{m}", name=f"e2n1T_{m}") for m in range(DT)]
            for mt in range(DT):
                pe = ps.tile([P, P], F32, tag="big", name="big", bufs=3)
                for kt in range(DT):
                    nc.tensor.matmul(
                        out=pe[:],
                        lhsT=w1s(DT + kt, mt),
                        rhs=featT[kt][:],
                        start=(kt == 0),
                        stop=(kt == DT - 1),
                    )
                nc.vector.tensor_scalar_add(
                    out=e2n1T[mt][:], in0=pe[:], scalar1=e_bias[mt][:, :1]
                )

            # ---- hT[dout, node] = W2a.T featT + W2b.T e2n1T + b2 ----
            hT = [sb.tile([P, P], F32, tag=f"hT_{m}", name=f"hT_{m}") for m in range(DT)]
            for mt in range(DT):
                ph = ps.tile([P, P], F32, tag="big", name="big", bufs=3)
                for kt in range(DT):
                    nc.tensor.matmul(
                        out=ph[:],
                        lhsT=w2s(kt, mt),
                        rhs=featT[kt][:],
                        start=(kt == 0),
                        stop=False,
                    )
                for kt in range(DT):
                    nc.tensor.matmul(
                        out=ph[:],
                        lhsT=w2s(DT + kt, mt),
                        rhs=e2n1T[kt][:],
                        start=False,
                        stop=(kt == DT - 1),
                    )
                nc.vector.tensor_scalar_add(
                    out=hT[mt][:], in0=ph[:], scalar1=b2s(mt)
                )

            # ---- w34 = W3 @ W4a  (512-vector), packed as [d,2] per d-tile ----
            w34cat = [sb.tile([P, 2], F32, tag=f"w34c_{d}", name=f"w34c_{d}") for d in range(DT)]
            for it in range(KT):
                pw = ps.tile([P, 1], F32, tag="small", name="small", bufs=2)
                for jt in range(DT):
                    nc.tensor.matmul(
                        out=pw[:],
                        lhsT=w3s(jt, it),
                        rhs=w4s(jt),
                        start=(jt == 0),
                        stop=(jt == DT - 1),
                    )
                d, col = it % DT, it // DT
                nc.vector.tensor_copy(out=w34cat[d][:, col : col + 1], in_=pw[:])

            # ---- c = b3 @ W4a + Xs @ W4b + b4   (scalar) ----
            pc = ps.tile([1, 1], F32, tag="small", name="small", bufs=2)
            nc.tensor.matmul(out=pc[:], lhsT=b3s(0), rhs=w4s(0), start=True, stop=False)
            nc.tensor.matmul(out=pc[:], lhsT=b3s(1), rhs=w4s(1), start=False, stop=False)
            nc.tensor.matmul(out=pc[:], lhsT=xss(0), rhs=w4s(2), start=False, stop=False)
            nc.tensor.matmul(out=pc[:], lhsT=xss(1), rhs=w4s(3), start=False, stop=True)
            c_sb = sb.tile([1, 1], F32, tag="c_sb", name="c_sb")
            nc.vector.tensor_add(out=c_sb[:], in0=pc[:], in1=b4s)

            # ---- p/q rows on partition 0 ----
            p_ps = ps.tile([1, P], F32, tag="pps", name="pps")
            q_ps = ps.tile([1, P], F32, tag="qps", name="qps")
            for d in range(DT):
                nc.tensor.matmul(
                    out=p_ps[:],
                    lhsT=w34cat[d][:, 0:1],
                    rhs=hT[d][:],
                    start=(d == 0),
                    stop=(d == DT - 1),
                )
            for d in range(DT):
                nc.tensor.matmul(
                    out=q_ps[:],
                    lhsT=w34cat[d][:, 1:2],
                    rhs=hT[d][:],
                    start=(d == 0),
                    stop=(d == DT - 1),
                )

            # ---- outer sum via two rank-1 matmuls, then sigmoid ----
            p_row = sb.tile([1, P], F32, tag="p_row", name="p_row")
            q_row = sb.tile([1, P], F32, tag="q_row", name="q_row")
            nc.vector.tensor_scalar_add(
                out=p_row[:], in0=p_ps[:], scalar1=c_sb[:1, :1]
            )
            nc.vector.tensor_copy(out=q_row[:], in_=q_ps[:])

            pz = ps.tile([P, P], F32, tag="big", name="big", bufs=3)
            nc.tensor.matmul(out=pz[:], lhsT=p_row[:], rhs=ones_row[:], start=True, stop=False)
            nc.tensor.matmul(out=pz[:], lhsT=ones_row[:], rhs=q_row[:], start=False, stop=True)

            osb = sb.tile([P, P], F32, tag="osb", name="osb")
            nc.scalar.activation(out=osb[:], in_=pz[:], func=TS.Sigmoid)
            nc.sync.dma_start(
                out=out[:, :].rearrange("(u v) o -> u (v o)", v=N), in_=osb[:]
            )

    nc.compile()
    return nc


def _pack_weights(W1, W2, W3, W4, b1, b2, b3, b4, Xs_b):
    pkv = np.zeros((P, PKC), dtype=np.float32)
    for kt in range(4):
        pkv[:, _W1 + kt * 256 : _W1 + (kt + 1) * 256] = W1[kt * P : (kt + 1) * P, :]
        pkv[:, _W2 + kt * 256 : _W2 + (kt + 1) * 256] = W2[kt * P : (kt + 1) * P, :]
        pkv[:, _W4 + kt] = W4[kt * P : (kt + 1) * P, 0]
    W3T = W3.T
    for jt in range(2):
        pkv[:, _W3T + jt * 512 : _W3T + (jt + 1) * 512] = W3T[jt * P : (jt + 1) * P, :]
        pkv[:, _B1 + jt] = b1[jt * P : (jt + 1) * P]
        pkv[:, _B2 + jt] = b2[jt * P : (jt + 1) * P]
        pkv[:, _B3 + jt] = b3[jt * P : (jt + 1) * P]
        pkv[:, _XS + jt] = Xs_b[jt * P : (jt + 1) * P]
    pkv[0, _B4] = b4[0]
    return pkv


def kernel(Xs, api_embeds, W1, b1, W2, b2, W3, b3, W4, b4, invoked):
    global LAST_RESULTS
    if "nc" not in _NC_CACHE:
        _NC_CACHE["nc"] = _build_nc()
    nc = _NC_CACHE["nc"]

    Xs = np.asarray(Xs, dtype=np.float32)
    emb = np.ascontiguousarray(np.asarray(api_embeds, dtype=np.float32))
    W1 = np.asarray(W1, dtype=np.float32)
    W2 = np.asarray(W2, dtype=np.float32)
    W3 = np.asarray(W3, dtype=np.float32)
    W4 = np.asarray(W4, dtype=np.float32).reshape(2 * D, 1)
    b1 = np.asarray(b1, dtype=np.float32).reshape(D)
    b2 = np.asarray(b2, dtype=np.float32).reshape(D)
    b3 = np.asarray(b3, dtype=np.float32).reshape(D)
    b4 = np.asarray(b4, dtype=np.float32).reshape(1)
    invoked = np.asarray(invoked, dtype=np.int32)

    in_maps = []
    for b in range(B):
        in_maps.append(
            {
                "invoked": np.ascontiguousarray(invoked[b].reshape(N, 1)),
                "emb": emb,
                "pk": _pack_weights(W1, W2, W3, W4, b1, b2, b3, b4, Xs[b]),
            }
        )

    res = run_bass_kernel_spmd(nc, in_maps, core_ids=list(range(B)), trace=TRACE)
    LAST_RESULTS = res
    return np.stack([res.results[i]["out"] for i in range(B)], axis=0)


# revision 9
# speedup vs baseline: 1.9700x; 1.4474x over previous
"""Trainium2 Bass kernel for nn_AdaptiveHyperNN (gnn_message_passing).

Math: the reference builds fully-connected edge features [B,N,N,D] through
linear layers, then mean-aggregates.  Because every edge MLP is linear
(no nonlinearity before aggregation), everything collapses:

  feat   = api_embeds[invoked]                       [B,N,D]
  e2n1_v = fbar @ W1a + feat_v @ W1b + b1            (fbar = mean_v feat_v)
  h_v    = feat_v @ W2a + e2n1_v @ W2b + b2
  logit[u*N+v] = p[u] + q[v] + c
     p = h @ (W3a @ W4a),  q = h @ (W3b @ W4a)
     c = b3 @ W4a + Xs @ W4b + b4
  out = sigmoid(logit) = sigmoid(q_outer + p_col_bias)  (fused in ACT)

Sharding: data-parallel over B (8 graphs -> 8 cores), weights replicated.
Each core: indirect-DMA gather of 128 embedding rows, ~30 matmuls (bf16),
rank-1 outer-sum matmul, sigmoid with per-partition bias, DMA out 64KB.

Perf structure: weights are host-packed bf16 into one [128, 3072] DRAM
tensor + a small f32 const block, loaded by large DMAs split across the
two HWDGE rings (sync + scalar engines).  The int32 `invoked` indices go
first so the gpsimd indirect gather starts immediately.  PSUM evictions
ride the vector engine; the scalar engine only ever runs Sigmoid (single
ACT table load, warmed up early).
"""

import numpy as np
import ml_dtypes

import concourse.bacc as bacc
import concourse.bass as bass
import concourse.mybir as mybir
import concourse.tile as tile
from concourse.bass import ts
from concourse.bass_utils import run_bass_kernel_spmd
from concourse.masks import make_identity

P = 128
D = 256
N = 128
B = 8
V = 10000
F32 = mybir.dt.float32
BF16 = mybir.dt.bfloat16
I32 = mybir.dt.int32

# packed bf16 weight layout [128, 3072]
_W1 = 0       # 4 k-tiles x 256
_W2 = 1024    # 4 k-tiles x 256
_W3T = 2048   # 2 j-tiles x 512
PKW = 3072
# packed f32 small block [128, 13]: w4(4) b1(2) b2(2) b3(2) xs(2) b4(1)
PKS = 13

TRACE = False
LAST_RESULTS = None

_NC_CACHE = {}


def _build_nc():
    nc = bacc.Bacc("TRN2", target_bir_lowering=False)

    inv = nc.dram_tensor("invoked", [N, 1], I32, kind="ExternalInput")
    emb = nc.dram_tensor("emb", [V, D], F32, kind="ExternalInput")
    pkw = nc.dram_tensor("pkw", [P, PKW], BF16, kind="ExternalInput")
    pks = nc.dram_tensor("pks", [P, PKS], F32, kind="ExternalInput")
    out = nc.dram_tensor("out", [N * N, 1], F32, kind="ExternalOutput")

    KT = 4
    DT = 2

    TS = mybir.ActivationFunctionType

    with tile.TileContext(nc) as tc:
        with (
            tc.tile_pool(name="sb", bufs=1) as sb,
            tc.tile_pool(name="ps", bufs=1, space="PSUM") as ps,
        ):
            # ---- indices first (scalar/ACT ring), gather ASAP ----
            inv_t = sb.tile([P, 1], I32, tag="inv", name="inv")
            nc.scalar.dma_start(out=inv_t[:], in_=inv[:, :])

            # sigmoid ACT-table warmup off the critical path
            warm = sb.tile([1, 1], F32, tag="warm", name="warm")
            warm2 = sb.tile([1, 1], F32, tag="warm2", name="warm2")
            nc.vector.memset(warm[:], 0.0)
            nc.scalar.activation(out=warm2[:], in_=warm[:], func=TS.Sigmoid)

            ident = sb.tile([P, P], F32, tag="ident", name="ident")
            make_identity(nc, ident[:])

            # ---- gather feat = emb[invoked]  [128 nodes, 256] ----
            feat = sb.tile([P, D], F32, tag="feat", name="feat")
            nc.gpsimd.indirect_dma_start(
                out=feat[:],
                out_offset=None,
                in_=emb[:, :],
                in_offset=bass.IndirectOffsetOnAxis(ap=inv_t[:, :1], axis=0),
            )

            # ---- packed weights: large DMAs over the two HWDGE rings ----
            sm_sb = sb.tile([P, PKS], F32, tag="sm", name="sm")
            wa_sb = sb.tile([P, 1024], BF16, tag="wa", name="wa")
            wb_sb = sb.tile([P, 1024], BF16, tag="wb", name="wb")
            wc_sb = sb.tile([P, 1024], BF16, tag="wc", name="wc")
            nc.scalar.dma_start(out=sm_sb[:], in_=pks[:, :])
            nc.sync.dma_start(out=wa_sb[:], in_=pkw[:, _W1:_W1 + 1024])
            nc.scalar.dma_start(out=wb_sb[:], in_=pkw[:, _W2:_W2 + 1024])
            nc.sync.dma_start(out=wc_sb[:], in_=pkw[:, _W3T:_W3T + 1024])

            def w1s(kt, mt):
                return wa_sb[:, kt * 256 + mt * 128 : kt * 256 + (mt + 1) * 128]

            def w2s(kt, mt):
                return wb_sb[:, kt * 256 + mt * 128 : kt * 256 + (mt + 1) * 128]

            def w3s(jt, it):
                return wc_sb[:, jt * 512 + it * 128 : jt * 512 + (it + 1) * 128]

            def w4s(kt):
                return sm_sb[:, kt : kt + 1]

            def b1s(k):
                return sm_sb[:, 4 + k : 5 + k]

            def b2s(k):
                return sm_sb[:, 6 + k : 7 + k]

            def b3s(k):
                return sm_sb[:, 8 + k : 9 + k]

            def xss(k):
                return sm_sb[:, 10 + k : 11 + k]

            b4s = sm_sb[0:1, 12:13]

            ones_row = sb.tile([1, P], BF16, tag="ones_row", name="ones_row")
            nc.vector.memset(ones_row[:], 1.0)

            # bf16 cast of the W4a columns (rhs of the w34 matmuls)
            w4bf = sb.tile([P, 2], BF16, tag="w4bf", name="w4bf")
            nc.vector.tensor_copy(out=w4bf[:], in_=sm_sb[:, 0:2])

            # ---- transpose -> featT (bf16) [din, node]; fbar = row sums ----
            featT = [sb.tile([P, P], BF16, tag=f"featT_{i}", name=f"featT_{i}") for i in range(DT)]
            fbar = [sb.tile([P, 1], BF16, tag=f"fbar_{i}", name=f"fbar_{i}") for i in range(DT)]
            fbar32 = [sb.tile([P, 1], F32, tag=f"fbar32_{i}", name=f"fbar32_{i}") for i in range(DT)]
            for i in range(DT):
                ptr = ps.tile([P, P], F32, tag="big", name="big", bufs=3)
                nc.tensor.transpose(out=ptr[:], in_=feat[:, ts(i, P)], identity=ident[:])
                nc.vector.tensor_copy(out=featT[i][:], in_=ptr[:])
                nc.vector.reduce_sum(
                    out=fbar32[i][:], in_=ptr[:], axis=mybir.AxisListType.X
                )
                nc.vector.tensor_copy(out=fbar[i][:], in_=fbar32[i][:])

            # ---- e_bias = (fbar_sum @ W1a)/128 + b1   [dout,1] per tile ----
            e_bias = [sb.tile([P, 1], F32, tag=f"ebias_{m}", name=f"ebias_{m}") for m in range(DT)]
            for mt in range(DT):
                pt1 = ps.tile([P, 1], F32, tag="small", name="small", bufs=2)
                for kt in range(DT):
                    nc.tensor.matmul(
                        out=pt1[:],
                        lhsT=w1s(kt, mt),
                        rhs=fbar[kt][:],
                        start=(kt == 0),
                        stop=(kt == DT - 1),
                    )
                nc.vector.tensor_scalar(
                    out=e_bias[mt][:],
                    in0=pt1[:],
                    scalar1=1.0 / N,
                    scalar2=b1s(mt),
                    op0=mybir.AluOpType.mult,
                    op1=mybir.AluOpType.add,
                )

            # ---- e2n1T[dout, node] = W1b.T-contract featT + e_bias ----
            e2n1T = [sb.tile([P, P], BF16, tag=f"e2n1T_{m}", name=f"e2n1T_{m}") for m in range(DT)]
            for mt in range(DT):
                pe = ps.tile([P, P], F32, tag="big", name="big", bufs=3)
                for kt in range(DT):
                    nc.tensor.matmul(
                        out=pe[:],
                        lhsT=w1s(DT + kt, mt),
                        rhs=featT[kt][:],
                        start=(kt == 0),
                        stop=(kt == DT - 1),
                    )
                nc.vector.tensor_scalar_add(
                    out=e2n1T[mt][:], in0=pe[:], scalar1=e_bias[mt][:, :1]
                )

            # ---- hT[dout, node] = W2a.T featT + W2b.T e2n1T + b2 ----
            hT = [sb.tile([P, P], BF16, tag=f"hT_{m}", name=f"hT_{m}") for m in range(DT)]
            for mt in range(DT):
                ph = ps.tile([P, P], F32, tag="big", name="big", bufs=3)
                for kt in range(DT):
                    nc.tensor.matmul(
                        out=ph[:],
                        lhsT=w2s(kt, mt),
                        rhs=featT[kt][:],
                        start=(kt == 0),
                        stop=False,
                    )
                for kt in range(DT):
                    nc.tensor.matmul(
                        out=ph[:],
                        lhsT=w2s(DT + kt, mt),
                        rhs=e2n1T[kt][:],
                        start=False,
                        stop=(kt == DT - 1),
                    )
                nc.vector.tensor_scalar_add(
                    out=hT[mt][:], in0=ph[:], scalar1=b2s(mt)
                )

            # ---- w34 = W3 @ W4a  (512-vector), packed as [d,2] per d-tile ----
            w34cat = [sb.tile([P, 2], BF16, tag=f"w34c_{d}", name=f"w34c_{d}") for d in range(DT)]
            for it in range(KT):
                pw = ps.tile([P, 1], F32, tag="small", name="small", bufs=2)
                for jt in range(DT):
                    nc.tensor.matmul(
                        out=pw[:],
                        lhsT=w3s(jt, it),
                        rhs=w4bf[:, jt : jt + 1],
                        start=(jt == 0),
                        stop=(jt == DT - 1),
                    )
                d, col = it % DT, it // DT
                nc.vector.tensor_copy(out=w34cat[d][:, col : col + 1], in_=pw[:])

            # ---- c = b3 @ W4a + Xs @ W4b + b4   (scalar, f32) ----
            pc = ps.tile([1, 1], F32, tag="small", name="small", bufs=2)
            nc.tensor.matmul(out=pc[:], lhsT=b3s(0), rhs=w4s(0), start=True, stop=False)
            nc.tensor.matmul(out=pc[:], lhsT=b3s(1), rhs=w4s(1), start=False, stop=False)
            nc.tensor.matmul(out=pc[:], lhsT=xss(0), rhs=w4s(2), start=False, stop=False)
            nc.tensor.matmul(out=pc[:], lhsT=xss(1), rhs=w4s(3), start=False, stop=True)
            c_sb = sb.tile([1, 1], F32, tag="c_sb", name="c_sb")
            nc.vector.tensor_add(out=c_sb[:], in0=pc[:], in1=b4s)

            # ---- p as column [node,1] (becomes the sigmoid bias), q as row ----
            p_ps = ps.tile([P, 1], F32, tag="pps", name="pps")
            q_ps = ps.tile([1, P], F32, tag="qps", name="qps")
            for d in range(DT):
                nc.tensor.matmul(
                    out=p_ps[:],
                    lhsT=hT[d][:],
                    rhs=w34cat[d][:, 0:1],
                    start=(d == 0),
                    stop=(d == DT - 1),
                )
            for d in range(DT):
                nc.tensor.matmul(
                    out=q_ps[:],
                    lhsT=w34cat[d][:, 1:2],
                    rhs=hT[d][:],
                    start=(d == 0),
                    stop=(d == DT - 1),
                )

            p_sb = sb.tile([P, 1], F32, tag="p_sb", name="p_sb")
            q_row = sb.tile([1, P], BF16, tag="q_row", name="q_row")
            nc.vector.tensor_copy(out=p_sb[:], in_=p_ps[:])
            nc.vector.tensor_scalar_add(
                out=q_row[:], in0=q_ps[:], scalar1=c_sb[:1, :1]
            )

            # ---- q broadcast over partitions, sigmoid(q + p) fused in ACT ----
            pz = ps.tile([P, P], F32, tag="big", name="big", bufs=3)
            nc.tensor.matmul(out=pz[:], lhsT=ones_row[:], rhs=q_row[:], start=True, stop=True)

            osb = sb.tile([P, P], F32, tag="osb", name="osb")
            nc.scalar.activation(
                out=osb[:], in_=pz[:], func=TS.Sigmoid, bias=p_sb[:, :1]
            )
            nc.sync.dma_start(
                out=out[:, :].rearrange("(u v) o -> u (v o)", v=N), in_=osb[:]
            )

    nc.compile()
    return nc


def _pack_w(W1, W2, W3):
    pkv = np.zeros((P, PKW), dtype=ml_dtypes.bfloat16)
    for kt in range(4):
        pkv[:, _W1 + kt * 256 : _W1 + (kt + 1) * 256] = W1[kt * P : (kt + 1) * P, :]
        pkv[:, _W2 + kt * 256 : _W2 + (kt + 1) * 256] = W2[kt * P : (kt + 1) * P, :]
    W3T = W3.T
    for jt in range(2):
        pkv[:, _W3T + jt * 512 : _W3T + (jt + 1) * 512] = W3T[jt * P : (jt + 1) * P, :]
    return pkv


def _pack_s(W4, b1, b2, b3, b4, Xs_b):
    pkv = np.zeros((P, PKS), dtype=np.float32)
    for kt in range(4):
        pkv[:, kt] = W4[kt * P : (kt + 1) * P, 0]
    for jt in range(2):
        pkv[:, 4 + jt] = b1[jt * P : (jt + 1) * P]
        pkv[:, 6 + jt] = b2[jt * P : (jt + 1) * P]
        pkv[:, 8 + jt] = b3[jt * P : (jt + 1) * P]
        pkv[:, 10 + jt] = Xs_b[jt * P : (jt + 1) * P]
    pkv[0, 12] = b4[0]
    return pkv


def kernel(Xs, api_embeds, W1, b1, W2, b2, W3, b3, W4, b4, invoked):
    global LAST_RESULTS
    if "nc" not in _NC_CACHE:
        _NC_CACHE["nc"] = _build_nc()
    nc = _NC_CACHE["nc"]

    Xs = np.asarray(Xs, dtype=np.float32)
    emb = np.ascontiguousarray(np.asarray(api_embeds, dtype=np.float32))
    W1 = np.asarray(W1, dtype=np.float32)
    W2 = np.asarray(W2, dtype=np.float32)
    W3 = np.asarray(W3, dtype=np.float32)
    W4 = np.asarray(W4, dtype=np.float32).reshape(2 * D, 1)
    b1 = np.asarray(b1, dtype=np.float32).reshape(D)
    b2 = np.asarray(b2, dtype=np.float32).reshape(D)
    b3 = np.asarray(b3, dtype=np.float32).reshape(D)
    b4 = np.asarray(b4, dtype=np.float32).reshape(1)
    invoked = np.asarray(invoked, dtype=np.int32)

    pkw = _pack_w(W1, W2, W3)
    in_maps = []
    for b in range(B):
        in_maps.append(
            {
                "invoked": np.ascontiguousarray(invoked[b].reshape(N, 1)),
                "emb": emb,
                "pkw": pkw,
                "pks": _pack_s(W4, b1, b2, b3, b4, Xs[b]),
            }
        )

    res = run_bass_kernel_spmd(nc, in_maps, core_ids=list(range(B)), trace=TRACE)
    LAST_RESULTS = res
    return np.stack([res.results[i]["out"] for i in range(B)], axis=0)
